# revision 10
# baseline (speedup 1.0000x reference)
"""TRN2 Bass kernel for nn_Attention_30485677867708.

Computes, for input [N=8192, D=256] and weights Q,K,V_down [D,H=128], V_up [H,D]:
    q = input @ Q; k = input @ K; v = input @ V_down
    attn = softmax(q @ k.T, axis=1)
    out  = (attn @ v) @ V_up            -> [N, D] fp32

Row-sharded SPMD over 8 NeuronCores (1024 rows each); K/V path replicated.

Per-core pipeline (all layouts chosen so softmax reductions are free-dim):
  prep: keyT/qT computed in fp16 hi/lo pairs (3-term matmuls == fp32-grade),
        v in fp16, all from host-supplied transposed fp16 hi/lo input.
  scores: S[rows,keys] chunks via 3-term fp16 matmuls into PSUM (fp32 accum)
  softmax: DVE chunk-max (negated) -> ACT exp(bias=-B_c, accum_out=sums_c)
           -> P fp16 in SBUF; per-row-tile epilogue folds exp(B_c - M)/rowsum
           into one per-chunk tensor_scalar rescale => P becomes attn weights.
  AV: PE-transpose P 128x128 blocks (packed 4-per-psum-tile), copy to SBUF,
      fp16 matmuls accumulate oT[H, rows]; transposes are kept outside open
      accumulation groups (hardware requirement) via explicit PE-order deps.
  out: dE[rows, D] = oT.T @ V_up in fp32, DMA out.
"""

import numpy as np
from contextlib import ExitStack

import concourse.bacc as bacc
from concourse import mybir
from concourse.tile import TileContext, add_dep_helper
from concourse.bass_utils import run_bass_kernel_spmd

f32 = mybir.dt.float32
f16 = mybir.dt.float16
EXP = mybir.ActivationFunctionType.Exp
MAX = mybir.AluOpType.max
AXX = mybir.AxisListType.X

N_CORES = 8


def build(N=8192, D=256, H=128, RPC=1024):
    """Build the per-core SPMD program. RPC = rows per core."""
    CHUNK = 1024                  # keys per softmax chunk (2 psum banks)
    NCH = N // CHUNK              # chunks per row tile
    RT = RPC // 128               # row tiles per core
    GRP = 2                       # row tiles per AV group
    NG = RT // GRP
    NKT = N // 128                # key tiles
    KB = 512                      # key block (matmul moving width)
    NKB = N // KB

    nc = bacc.Bacc("TRN2", target_bir_lowering=False)

    inh = nc.dram_tensor("inh", [D, N], f16, kind="ExternalInput")
    inl = nc.dram_tensor("inl", [D, N], f16, kind="ExternalInput")
    qih = nc.dram_tensor("qih", [D, RPC], f16, kind="ExternalInput")
    qil = nc.dram_tensor("qil", [D, RPC], f16, kind="ExternalInput")
    # [Qh | Ql | Kh | Kl | Vdh] each [D, H]
    wpk = nc.dram_tensor("wpk", [D, 5 * H], f16, kind="ExternalInput")
    vup = nc.dram_tensor("vup", [H, D], f32, kind="ExternalInput")
    idin = nc.dram_tensor("idin", [128, 128], f16, kind="ExternalInput")
    out = nc.dram_tensor("out", [RPC, D], f32, kind="ExternalOutput")

    # --- PE program-order fence: transposes must never land inside an open
    # PSUM accumulation group (hardware crash), so serialize all PE work in
    # emission order.
    pe_last = [None]

    def fence(inst):
        if pe_last[0] is not None:
            add_dep_helper(inst.ins, pe_last[0].ins, sync=False, reason="pe-order")
        pe_last[0] = inst
        return inst

    def mm(*args, **kw):
        return fence(nc.tensor.matmul(*args, **kw))

    with TileContext(nc) as tc, ExitStack() as ctx:
        wp = ctx.enter_context(tc.tile_pool(name="wp", bufs=1))
        big = ctx.enter_context(tc.tile_pool(name="big", bufs=1))

        wp0 = wp.tile([128, 5 * H], f16, tag="wp0")
        wp1 = wp.tile([128, 5 * H], f16, tag="wp1")
        vu = wp.tile([H, D], f32, tag="vu")
        ident = wp.tile([128, 128], f16, tag="ident")
        nc.sync.dma_start(wp0[:], wpk[0:128, :])
        nc.sync.dma_start(wp1[:], wpk[128:256, :])
        nc.sync.dma_start(vu[:], vup[:])
        nc.sync.dma_start(ident[:], idin[:])

        kh = big.tile([128, N], f16, tag="kh")
        kl = big.tile([128, N], f16, tag="kl")
        vsb = big.tile([128, N], f16, tag="vsb")
        qh = big.tile([128, RPC], f16, tag="qh")
        ql = big.tile([128, RPC], f16, tag="ql")

        # ---------------- prep ----------------
        with ExitStack() as prep:
            ipool = prep.enter_context(tc.tile_pool(name="ipool", bufs=1))
            pps = prep.enter_context(tc.tile_pool(name="pps", bufs=2, space="PSUM"))

            ih = [
                ipool.tile([128, N], f16, tag=f"ih{c}", name=f"ih{c}")
                for c in range(2)
            ]
            il = [
                ipool.tile([128, N], f16, tag=f"il{c}", name=f"il{c}")
                for c in range(2)
            ]
            qh_in = [
                ipool.tile([128, RPC], f16, tag=f"qhin{c}", name=f"qhin{c}")
                for c in range(2)
            ]
            ql_in = [
                ipool.tile([128, RPC], f16, tag=f"qlin{c}", name=f"qlin{c}")
                for c in range(2)
            ]
            for c in range(2):
                sl = slice(c * 128, (c + 1) * 128)
                nc.sync.dma_start(ih[c][:], inh[sl, :])
                nc.sync.dma_start(il[c][:], inl[sl, :])
                nc.sync.dma_start(qh_in[c][:], qih[sl, :])
                nc.sync.dma_start(ql_in[c][:], qil[sl, :])

            wslice = lambda c, i: (wp0 if c == 0 else wp1)[:, i * H : (i + 1) * H]

            def hilo3(ps_ap, w_hi_i, w_lo_i, mov_h, mov_l):
                # psum += Wh.T@mh + Wh.T@ml + Wl.T@mh  over both D-chunks
                for c in range(2):
                    mm(ps_ap, wslice(c, w_hi_i), mov_h[c], start=(c == 0), stop=False)
                    mm(ps_ap, wslice(c, w_hi_i), mov_l[c], start=False, stop=False)
                    mm(ps_ap, wslice(c, w_lo_i), mov_h[c], start=False,
                       stop=(c == 1))

            # keyT -> kh/kl fp16 pair
            for b in range(NKB):
                ks = slice(b * KB, (b + 1) * KB)
                pk = pps.tile([128, KB], f32, tag="pk")
                hilo3(pk[:], 2, 3, [t[:, ks] for t in ih], [t[:, ks] for t in il])
                nc.scalar.copy(kh[:, ks], pk[:])
                nc.vector.tensor_sub(kl[:, ks], pk[:], kh[:, ks])

            # qT -> qh/ql fp16 pair
            QB = min(KB, RPC)
            for b in range(RPC // QB):
                rs = slice(b * QB, (b + 1) * QB)
                pq = pps.tile([128, QB], f32, tag="pq")
                hilo3(pq[:], 0, 1, [t[:, rs] for t in qh_in], [t[:, rs] for t in ql_in])
                nc.scalar.copy(qh[:, rs], pq[:])
                nc.vector.tensor_sub(ql[:, rs], pq[:], qh[:, rs])

            # v fp16 (keys on partitions): v[kt] = sum_c inh[c][:,kt].T @ Vdh[c]
            for b4 in range(NKT // 4):
                pv = pps.tile([128, 512], f32, tag="pv")
                for j in range(4):
                    kt = b4 * 4 + j
                    kslc = slice(kt * 128, (kt + 1) * 128)
                    for c in range(2):
                        mm(
                            pv[:, j * 128 : (j + 1) * 128],
                            ih[c][:, kslc],
                            wslice(c, 4),
                            start=(c == 0),
                            stop=(c == 1),
                            skip_group_check=True,
                        )
                nc.scalar.copy(vsb[:, b4 * 512 : (b4 + 1) * 512], pv[:])

        # ---------------- main ----------------
        # PSUM budget (8 banks): spsum 2x2 + tpsum 2x1 + opsum 1x1 + dpsum 1x1
        ppool = ctx.enter_context(tc.tile_pool(name="ppool", bufs=2 * GRP + 1))
        smalls = ctx.enter_context(tc.tile_pool(name="smalls", bufs=4))
        ptsb = ctx.enter_context(tc.tile_pool(name="ptsb", bufs=2 * NKT // 4 + 2))
        ostr = ctx.enter_context(tc.tile_pool(name="ostr", bufs=2))
        spsum = ctx.enter_context(tc.tile_pool(name="spsum", bufs=2, space="PSUM"))
        tpsum = ctx.enter_context(tc.tile_pool(name="tpsum", bufs=2, space="PSUM"))
        opsum = ctx.enter_context(tc.tile_pool(name="opsum", bufs=1, space="PSUM"))
        dpsum = ctx.enter_context(tc.tile_pool(name="dpsum", bufs=1, space="PSUM"))

        P_tiles = {}

        def softmax_tile(rt):
            """Scores + exp for row tile rt; P becomes attn weights (fp16)."""
            P = ppool.tile([128, N], f16, tag="P")
            P_tiles[rt] = P
            negB = smalls.tile([128, NCH], f32, tag="negB")
            sums = smalls.tile([128, NCH], f32, tag="sums")
            lh = qh[:, rt * 128 : (rt + 1) * 128]
            ll = ql[:, rt * 128 : (rt + 1) * 128]
            for c in range(NCH):
                ps = spsum.tile([128, CHUNK], f32, tag="ps")
                for hblk in range(CHUNK // KB):
                    o = ps[:, hblk * KB : (hblk + 1) * KB]
                    ks = slice(c * CHUNK + hblk * KB, c * CHUNK + (hblk + 1) * KB)
                    mm(o, lh, kh[:, ks], start=True, stop=False)
                    mm(o, lh, kl[:, ks], start=False, stop=False)
                    mm(o, ll, kh[:, ks], start=False, stop=True)
                nc.vector.tensor_reduce(
                    negB[:, c : c + 1], ps[:], axis=AXX, op=MAX, negate=True
                )
                nc.scalar.activation(
                    P[:, c * CHUNK : (c + 1) * CHUNK],
                    ps[:],
                    EXP,
                    bias=negB[:, c : c + 1],
                    scale=1.0,
                    accum_out=sums[:, c : c + 1],
                )
            # epilogue: G_c = exp(B_c - M) / sum_r ; P *= G_c
            negM = smalls.tile([128, 1], f32, tag="negM")
            # negB holds -B_c; M = max_c B_c  =>  -M = min_c(-B_c)... via max on
            # negated: reduce max of negB gives -min(B); we need min of negB.
            nc.vector.tensor_reduce(
                negM[:], negB[:], axis=AXX, op=mybir.AluOpType.min
            )
            F = smalls.tile([128, NCH], f32, tag="F")
            # F_c = exp(B_c - M) = exp(-(negB_c) + negM) -> scale=-1, bias=negM
            nc.scalar.activation(F[:], negB[:], EXP, bias=negM[:], scale=-1.0)
            T = smalls.tile([128, NCH], f32, tag="T")
            nc.vector.tensor_mul(T[:], F[:], sums[:])
            S = smalls.tile([128, 1], f32, tag="S")
            nc.vector.tensor_reduce(S[:], T[:], axis=AXX, op=mybir.AluOpType.add)
            R = smalls.tile([128, 1], f32, tag="R")
            nc.vector.reciprocal(R[:], S[:])
            G = smalls.tile([128, NCH], f32, tag="G")
            nc.vector.tensor_scalar_mul(G[:], F[:], R[:])
            for c in range(NCH):
                sl = slice(c * CHUNK, (c + 1) * CHUNK)
                nc.vector.tensor_scalar_mul(P[:, sl], P[:, sl], G[:, c : c + 1])
            return P

        def av_half(g, half, oacc):
            """Transpose+copy batch for 32 kts, then the AV matmul run."""
            kts = range(half * NKT // 2, (half + 1) * NKT // 2)
            slabs = []
            for i, kt in enumerate(kts):
                if i % 2 == 0:
                    pt = tpsum.tile([128, 512], f16, tag="pt")
                    pts = ptsb.tile([128, 512], f16, tag="pts")
                    slabs.append(pts)
                for j in range(GRP):
                    quad = (i % 2) * GRP + j
                    mm(
                        pt[:, quad * 128 : (quad + 1) * 128],
                        P_tiles[g * GRP + j][:, kt * 128 : (kt + 1) * 128],
                        ident[:],
                        is_transpose=True,
                        skip_group_check=True,
                    )
                if i % 2 == 1:
                    if (kt // 2) % 2 == 0:
                        nc.vector.tensor_copy(pts[:], pt[:])
                    else:
                        nc.scalar.copy(pts[:], pt[:])
            nkts = len(list(kts))
            for i, kt in enumerate(kts):
                pts = slabs[i // 2]
                rhs = pts[:, (i % 2) * GRP * 128 : ((i % 2) + 1) * GRP * 128]
                mm(
                    oacc[:],
                    vsb[:, kt * 128 : (kt + 1) * 128],
                    rhs,
                    start=(i == 0),
                    stop=(i == nkts - 1),
                )

        def av_group(g):
            # one psum bank holds both half-accumulators side by side
            oab = opsum.tile([128, 2 * GRP * 128], f32, tag="oab")
            av_half(g, 0, oab[:, 0 : GRP * 128])
            av_half(g, 1, oab[:, GRP * 128 : 2 * GRP * 128])
            oTs = ostr.tile([128, GRP * 128], f32, tag="oTs")
            nc.scalar.copy(oTs[:], oab[:, 0 : GRP * 128])
            nc.vector.tensor_add(
                oTs[:], oTs[:], oab[:, GRP * 128 : 2 * GRP * 128]
            )
            for j in range(GRP):
                rt = g * GRP + j
                pd = dpsum.tile([128, D], f32, tag="pd")
                mm(pd[:], oTs[:, j * 128 : (j + 1) * 128], vu[:], start=True, stop=True)
                dEs = ostr.tile([128, D], f32, tag="dEs")
                nc.scalar.copy(dEs[:], pd[:])
                nc.sync.dma_start(out[rt * 128 : (rt + 1) * 128, :], dEs[:])

        # emission order: first 2 groups of softmax, then interleave AV of
        # finished groups with remaining softmax tiles to keep all engines fed.
        lead = min(RT, 2 * GRP)
        for rt in range(lead):
            softmax_tile(rt)
        done = 0
        for rt in range(lead, RT):
            softmax_tile(rt)
            av_group(done)
            done += 1
        while done < NG:
            av_group(done)
            done += 1

    return nc


def _split16(x):
    hi = x.astype(np.float16)
    lo = (x - hi.astype(np.float32)).astype(np.float16)
    return hi, lo


def kernel(input, Q, K, V_down, V_up):
    input = np.asarray(input, np.float32)
    Q = np.asarray(Q, np.float32)
    K = np.asarray(K, np.float32)
    V_down = np.asarray(V_down, np.float32)
    V_up = np.asarray(V_up, np.float32)

    N, D = input.shape
    H = Q.shape[1]
    RPC = N // N_CORES

    inT = np.ascontiguousarray(input.T)  # [D, N]
    inh, inl = _split16(inT)
    Qh, Ql = _split16(Q)
    Kh, Kl = _split16(K)
    Vdh = V_down.astype(np.float16)
    wpk = np.ascontiguousarray(
        np.concatenate([Qh, Ql, Kh, Kl, Vdh], axis=1)
    )
    ident = np.eye(128, dtype=np.float16)

    nc = build(N=N, D=D, H=H, RPC=RPC)
    nc.finalize()

    in_maps = []
    for c in range(N_CORES):
        sl = slice(c * RPC, (c + 1) * RPC)
        in_maps.append(
            {
                "inh": inh,
                "inl": inl,
                "qih": np.ascontiguousarray(inh[:, sl]),
                "qil": np.ascontiguousarray(inl[:, sl]),
                "wpk": wpk,
                "vup": V_up,
                "idin": ident,
            }
        )

    res = run_bass_kernel_spmd(nc, in_maps, core_ids=list(range(N_CORES)))
    return np.concatenate([res.results[c]["out"] for c in range(N_CORES)], axis=0)


# revision 11
# speedup vs baseline: 1.7063x; 1.7063x over previous
"""TRN2 Bass kernel for nn_Attention_30485677867708.

Computes, for input [N=8192, D=256] and weights Q,K,V_down [D,H=128], V_up [H,D]:
    q = input @ Q; k = input @ K; v = input @ V_down
    attn = softmax(q @ k.T, axis=1)
    out  = (attn @ v) @ V_up            -> [N, D] fp32

Row-sharded SPMD over 8 NeuronCores (1024 rows each); K/V path replicated.

Per-core pipeline:
  prep: keyT/qT computed in fp16 hi/lo pairs (3-term matmuls == fp32-grade
        scores), v in fp16, from host-supplied transposed fp16 hi/lo input.
  scores: S[rows,keys] 1024-key chunks via 3-term fp16 matmuls into PSUM.
  softmax: DVE chunk-max (negated) -> ACT exp(bias=-B_c, accum_out=sums_c)
           -> P fp16 in SBUF; per-row-tile epilogue folds exp(B_c - M)/rowsum
           into one per-chunk tensor_scalar rescale => P becomes attn weights.
  AV: P^T produced by DMA xbar transposes (SBUF->SBUF, batched 128x128
      blocks, two row-tiles packed side by side), then fp16 matmuls
      accumulate oT[H, rows] in two psum half-groups.
  out: dE[rows, D] = oT.T @ V_up in fp32, DMA out.
"""

import numpy as np
from contextlib import ExitStack

import concourse.bacc as bacc
from concourse import mybir
from concourse.tile import TileContext, add_dep_helper
from concourse.bass_utils import run_bass_kernel_spmd

f32 = mybir.dt.float32
f16 = mybir.dt.float16
EXP = mybir.ActivationFunctionType.Exp
MAX = mybir.AluOpType.max
AXX = mybir.AxisListType.X

N_CORES = 8


def build(N=8192, D=256, H=128, RPC=1024):
    """Build the per-core SPMD program. RPC = rows per core."""
    CHUNK = 1024                  # keys per softmax chunk (2 psum banks)
    NCH = N // CHUNK
    RT = RPC // 128               # row tiles per core
    GRP = 2                       # row tiles per AV group
    NG = RT // GRP
    NKT = N // 128                # key tiles
    HKT = NKT // 2                # key tiles per AV half
    KB = 512                      # matmul moving width
    NKB = N // KB

    nc = bacc.Bacc("TRN2", target_bir_lowering=False)

    inh = nc.dram_tensor("inh", [D, N], f16, kind="ExternalInput")
    inl = nc.dram_tensor("inl", [D, N], f16, kind="ExternalInput")
    qih = nc.dram_tensor("qih", [D, RPC], f16, kind="ExternalInput")
    qil = nc.dram_tensor("qil", [D, RPC], f16, kind="ExternalInput")
    # [Qh | Ql | Kh | Kl | Vdh] each [D, H]
    wpk = nc.dram_tensor("wpk", [D, 5 * H], f16, kind="ExternalInput")
    vup = nc.dram_tensor("vup", [H, D], f32, kind="ExternalInput")
    out = nc.dram_tensor("out", [RPC, D], f32, kind="ExternalOutput")

    # PE program-order fence (keeps accumulation groups clean and makes the
    # PE schedule exactly the emission order).
    pe_last = [None]

    def mm(*args, **kw):
        inst = nc.tensor.matmul(*args, **kw)
        if pe_last[0] is not None:
            add_dep_helper(inst.ins, pe_last[0].ins, sync=False, reason="pe-order")
        pe_last[0] = inst
        return inst

    with TileContext(nc) as tc, ExitStack() as ctx:
        wp = ctx.enter_context(tc.tile_pool(name="wp", bufs=1))
        big = ctx.enter_context(tc.tile_pool(name="big", bufs=1))

        wp0 = wp.tile([128, 5 * H], f16, tag="wp0")
        wp1 = wp.tile([128, 5 * H], f16, tag="wp1")
        vu = wp.tile([H, D], f32, tag="vu")
        nc.sync.dma_start(wp0[:], wpk[0:128, :])
        nc.sync.dma_start(wp1[:], wpk[128:256, :])
        nc.sync.dma_start(vu[:], vup[:])

        kh = big.tile([128, N], f16, tag="kh")
        kl = big.tile([128, N], f16, tag="kl")
        vsb = big.tile([128, N], f16, tag="vsb")
        qh = big.tile([128, RPC], f16, tag="qh")
        ql = big.tile([128, RPC], f16, tag="ql")

        # ---------------- prep ----------------
        with ExitStack() as prep:
            ipool = prep.enter_context(tc.tile_pool(name="ipool", bufs=1))
            pps = prep.enter_context(tc.tile_pool(name="pps", bufs=2, space="PSUM"))

            qh_in = [
                ipool.tile([128, RPC], f16, tag=f"qhin{c}", name=f"qhin{c}")
                for c in range(2)
            ]
            ql_in = [
                ipool.tile([128, RPC], f16, tag=f"qlin{c}", name=f"qlin{c}")
                for c in range(2)
            ]
            ih = [
                ipool.tile([128, N], f16, tag=f"ih{c}", name=f"ih{c}")
                for c in range(2)
            ]
            il = [
                ipool.tile([128, N], f16, tag=f"il{c}", name=f"il{c}")
                for c in range(2)
            ]
            for c in range(2):
                sl = slice(c * 128, (c + 1) * 128)
                nc.sync.dma_start(qh_in[c][:], qih[sl, :])
                nc.sync.dma_start(ql_in[c][:], qil[sl, :])
            # big input DMAs sliced so prep matmuls start early
            DSL = 2048
            for j in range(N // DSL):
                js = slice(j * DSL, (j + 1) * DSL)
                for c in range(2):
                    sl = slice(c * 128, (c + 1) * 128)
                    nc.sync.dma_start(ih[c][:, js], inh[sl, js])
                    nc.sync.dma_start(il[c][:, js], inl[sl, js])

            wslice = lambda c, i: (wp0 if c == 0 else wp1)[:, i * H : (i + 1) * H]

            def hilo3(ps_ap, w_hi_i, w_lo_i, mov_h, mov_l):
                for c in range(2):
                    mm(ps_ap, wslice(c, w_hi_i), mov_h[c], start=(c == 0), stop=False)
                    mm(ps_ap, wslice(c, w_hi_i), mov_l[c], start=False, stop=False)
                    mm(ps_ap, wslice(c, w_lo_i), mov_h[c], start=False,
                       stop=(c == 1))

            # qT -> qh/ql fp16 pair (small; first, while big DMAs stream)
            QB = min(KB, RPC)
            for b in range(RPC // QB):
                rs = slice(b * QB, (b + 1) * QB)
                pq = pps.tile([128, QB], f32, tag="pq")
                hilo3(pq[:], 0, 1, [t[:, rs] for t in qh_in], [t[:, rs] for t in ql_in])
                nc.scalar.copy(qh[:, rs], pq[:])
                nc.vector.tensor_sub(ql[:, rs], pq[:], qh[:, rs])

            # keyT -> kh/kl fp16 pair
            for b in range(NKB):
                ks = slice(b * KB, (b + 1) * KB)
                pk = pps.tile([128, KB], f32, tag="pk")
                hilo3(pk[:], 2, 3, [t[:, ks] for t in ih], [t[:, ks] for t in il])
                nc.scalar.copy(kh[:, ks], pk[:])
                nc.vector.tensor_sub(kl[:, ks], pk[:], kh[:, ks])

            # v fp16 (keys on partitions)
            for b4 in range(NKT // 4):
                pv = pps.tile([128, 512], f32, tag="pv")
                for j in range(4):
                    kt = b4 * 4 + j
                    kslc = slice(kt * 128, (kt + 1) * 128)
                    for c in range(2):
                        mm(
                            pv[:, j * 128 : (j + 1) * 128],
                            ih[c][:, kslc],
                            wslice(c, 4),
                            start=(c == 0),
                            stop=(c == 1),
                            skip_group_check=True,
                        )
                nc.scalar.copy(vsb[:, b4 * 512 : (b4 + 1) * 512], pv[:])

        # ---------------- main ----------------
        # PSUM budget (8 banks): spsum 3x2 + opsum 1 + dpsum 1
        ppool = ctx.enter_context(tc.tile_pool(name="ppool", bufs=2 * GRP))
        smalls = ctx.enter_context(tc.tile_pool(name="smalls", bufs=4))
        ptsb = ctx.enter_context(tc.tile_pool(name="ptsb", bufs=3))
        ostr = ctx.enter_context(tc.tile_pool(name="ostr", bufs=2))
        spsum = ctx.enter_context(tc.tile_pool(name="spsum", bufs=3, space="PSUM"))
        opsum = ctx.enter_context(tc.tile_pool(name="opsum", bufs=1, space="PSUM"))
        dpsum = ctx.enter_context(tc.tile_pool(name="dpsum", bufs=1, space="PSUM"))

        P_tiles = {}
        pts_tiles = {}

        def softmax_tile(rt):
            P = ppool.tile([128, N], f16, tag="P")
            P_tiles[rt] = P
            negB = smalls.tile([128, NCH], f32, tag="negB")
            sums = smalls.tile([128, NCH], f32, tag="sums")
            lh = qh[:, rt * 128 : (rt + 1) * 128]
            ll = ql[:, rt * 128 : (rt + 1) * 128]
            for c in range(NCH):
                ps = spsum.tile([128, CHUNK], f32, tag="ps")
                for hblk in range(CHUNK // KB):
                    o = ps[:, hblk * KB : (hblk + 1) * KB]
                    ks = slice(c * CHUNK + hblk * KB, c * CHUNK + (hblk + 1) * KB)
                    mm(o, lh, kh[:, ks], start=True, stop=False)
                    mm(o, lh, kl[:, ks], start=False, stop=False)
                    mm(o, ll, kh[:, ks], start=False, stop=True)
                nc.vector.tensor_reduce(
                    negB[:, c : c + 1], ps[:], axis=AXX, op=MAX, negate=True
                )
                nc.scalar.activation(
                    P[:, c * CHUNK : (c + 1) * CHUNK],
                    ps[:],
                    EXP,
                    bias=negB[:, c : c + 1],
                    scale=1.0,
                    accum_out=sums[:, c : c + 1],
                )
            negM = smalls.tile([128, 1], f32, tag="negM")
            nc.vector.tensor_reduce(
                negM[:], negB[:], axis=AXX, op=mybir.AluOpType.min
            )
            F = smalls.tile([128, NCH], f32, tag="F")
            nc.scalar.activation(F[:], negB[:], EXP, bias=negM[:], scale=-1.0)
            T = smalls.tile([128, NCH], f32, tag="T")
            nc.vector.tensor_mul(T[:], F[:], sums[:])
            S = smalls.tile([128, 1], f32, tag="S")
            nc.vector.tensor_reduce(S[:], T[:], axis=AXX, op=mybir.AluOpType.add)
            R = smalls.tile([128, 1], f32, tag="R")
            nc.vector.reciprocal(R[:], S[:])
            G = smalls.tile([128, NCH], f32, tag="G")
            nc.vector.tensor_scalar_mul(G[:], F[:], R[:])
            for c in range(NCH):
                sl = slice(c * CHUNK, (c + 1) * CHUNK)
                nc.vector.tensor_scalar_mul(P[:, sl], P[:, sl], G[:, c : c + 1])

        def transpose_group(g):
            """DMA xbar transposes: P^T half-tiles [128, HKT, GRP*128]."""
            for h in range(2):
                pts = ptsb.tile([128, HKT, GRP * 128], f16, tag="pts")
                pts_tiles[(g, h)] = pts
                for j in range(GRP):
                    rt = g * GRP + j
                    nc.sync.dma_start(
                        pts[:][:, :, j * 128 : (j + 1) * 128],
                        P_tiles[rt][:, h * (N // 2) : (h + 1) * (N // 2)],
                        transpose=True,
                    )

        def av_group(g):
            oab = opsum.tile([128, 2 * GRP * 128], f32, tag="oab")
            for h in range(2):
                oacc = oab[:, h * GRP * 128 : (h + 1) * GRP * 128]
                pts = pts_tiles.pop((g, h))
                for i in range(HKT):
                    kt = h * HKT + i
                    mm(
                        oacc,
                        vsb[:, kt * 128 : (kt + 1) * 128],
                        pts[:][:, i, :],
                        start=(i == 0),
                        stop=(i == HKT - 1),
                    )
            oTs = ostr.tile([128, GRP * 128], f32, tag="oTs")
            nc.scalar.copy(oTs[:], oab[:, 0 : GRP * 128])
            nc.vector.tensor_add(
                oTs[:], oTs[:], oab[:, GRP * 128 : 2 * GRP * 128]
            )
            for j in range(GRP):
                rt = g * GRP + j
                pd = dpsum.tile([128, D], f32, tag="pd")
                mm(pd[:], oTs[:, j * 128 : (j + 1) * 128], vu[:], start=True, stop=True)
                dEs = ostr.tile([128, D], f32, tag="dEs")
                nc.scalar.copy(dEs[:], pd[:])
                nc.sync.dma_start(out[rt * 128 : (rt + 1) * 128, :], dEs[:])

        for rt in range(RT):
            softmax_tile(rt)
            if rt % GRP == GRP - 1:
                g = rt // GRP
                transpose_group(g)
                if g >= 1:
                    av_group(g - 1)
        av_group(NG - 1)

    return nc


def _split16(x):
    hi = x.astype(np.float16)
    lo = (x - hi.astype(np.float32)).astype(np.float16)
    return hi, lo


def kernel(input, Q, K, V_down, V_up):
    input = np.asarray(input, np.float32)
    Q = np.asarray(Q, np.float32)
    K = np.asarray(K, np.float32)
    V_down = np.asarray(V_down, np.float32)
    V_up = np.asarray(V_up, np.float32)

    N, D = input.shape
    H = Q.shape[1]
    RPC = N // N_CORES

    inT = np.ascontiguousarray(input.T)  # [D, N]
    inh, inl = _split16(inT)
    Qh, Ql = _split16(Q)
    Kh, Kl = _split16(K)
    Vdh = V_down.astype(np.float16)
    wpk = np.ascontiguousarray(np.concatenate([Qh, Ql, Kh, Kl, Vdh], axis=1))

    nc = build(N=N, D=D, H=H, RPC=RPC)
    nc.finalize()

    in_maps = []
    for c in range(N_CORES):
        sl = slice(c * RPC, (c + 1) * RPC)
        in_maps.append(
            {
                "inh": inh,
                "inl": inl,
                "qih": np.ascontiguousarray(inh[:, sl]),
                "qil": np.ascontiguousarray(inl[:, sl]),
                "wpk": wpk,
                "vup": V_up,
            }
        )

    res = run_bass_kernel_spmd(nc, in_maps, core_ids=list(range(N_CORES)))
    return np.concatenate([res.results[c]["out"] for c in range(N_CORES)], axis=0)


# revision 16
# speedup vs baseline: 1.7311x; 1.0146x over previous
"""TRN2 Bass kernel for nn_Attention_30485677867708.

Computes, for input [N=8192, D=256] and weights Q,K,V_down [D,H=128], V_up [H,D]:
    q = input @ Q; k = input @ K; v = input @ V_down
    attn = softmax(q @ k.T, axis=1)
    out  = (attn @ v) @ V_up            -> [N, D] fp32

Row-sharded SPMD over 8 NeuronCores (1024 rows each); K/V path replicated.

Per-core pipeline:
  prep: keyT/qT computed in fp16 hi/lo pairs (3-term matmuls == fp32-grade
        scores), v in fp16, from host-supplied transposed fp16 hi/lo input.
  scores: S[rows,keys] 1024-key chunks via 3-term fp16 matmuls into PSUM.
  softmax: DVE chunk-max (negated) -> ACT exp(bias=-B_c, accum_out=sums_c)
           -> P fp16 in SBUF; per-row-tile epilogue folds exp(B_c - M)/rowsum
           into one per-chunk tensor_scalar rescale => P becomes attn weights.
  AV: P^T produced by DMA xbar transposes (SBUF->SBUF, batched 128x128
      blocks, two row-tiles packed side by side), then fp16 matmuls
      accumulate oT[H, rows] in two psum half-groups.
  out: dE[rows, D] = oT.T @ V_up in fp32, DMA out.
"""

import numpy as np
from contextlib import ExitStack

import concourse.bacc as bacc
from concourse import mybir
from concourse.tile import TileContext, add_dep_helper
from concourse.bass_utils import run_bass_kernel_spmd

f32 = mybir.dt.float32
f16 = mybir.dt.float16
EXP = mybir.ActivationFunctionType.Exp
MAX = mybir.AluOpType.max
AXX = mybir.AxisListType.X

N_CORES = 8


def build(N=8192, D=256, H=128, RPC=1024):
    """Build the per-core SPMD program. RPC = rows per core."""
    CHUNK = 1024                  # keys per softmax chunk (2 psum banks)
    NCH = N // CHUNK
    RT = RPC // 128               # row tiles per core
    GRP = 2                       # row tiles per AV group
    NG = RT // GRP
    NKT = N // 128                # key tiles
    HKT = NKT // 2                # key tiles per AV half
    KB = 512                      # matmul moving width
    NKB = N // KB

    nc = bacc.Bacc("TRN2", target_bir_lowering=False)

    inh = nc.dram_tensor("inh", [D, N], f16, kind="ExternalInput")
    inl = nc.dram_tensor("inl", [D, N], f16, kind="ExternalInput")
    qih = nc.dram_tensor("qih", [D, RPC], f16, kind="ExternalInput")
    qil = nc.dram_tensor("qil", [D, RPC], f16, kind="ExternalInput")
    # [Qh | Ql | Kh | Kl | Vdh] each [D, H]
    wpk = nc.dram_tensor("wpk", [D, 5 * H], f16, kind="ExternalInput")
    vup = nc.dram_tensor("vup", [H, D], f32, kind="ExternalInput")
    out = nc.dram_tensor("out", [RPC, D], f32, kind="ExternalOutput")

    # PE program-order fence (keeps accumulation groups clean and makes the
    # PE schedule exactly the emission order).
    pe_last = [None]

    def mm(*args, **kw):
        inst = nc.tensor.matmul(*args, **kw)
        if pe_last[0] is not None:
            add_dep_helper(inst.ins, pe_last[0].ins, sync=False, reason="pe-order")
        pe_last[0] = inst
        return inst

    with TileContext(nc) as tc, ExitStack() as ctx:
        wp = ctx.enter_context(tc.tile_pool(name="wp", bufs=1))
        big = ctx.enter_context(tc.tile_pool(name="big", bufs=1))

        wp0 = wp.tile([128, 5 * H], f16, tag="wp0")
        wp1 = wp.tile([128, 5 * H], f16, tag="wp1")
        vu = wp.tile([H, D], f32, tag="vu")
        nc.sync.dma_start(wp0[:], wpk[0:128, :])
        nc.sync.dma_start(wp1[:], wpk[128:256, :])
        nc.sync.dma_start(vu[:], vup[:])

        kh = big.tile([128, N], f16, tag="kh")
        kl = big.tile([128, N], f16, tag="kl")
        vsb = big.tile([128, N], f16, tag="vsb")
        qh = big.tile([128, RPC], f16, tag="qh")
        ql = big.tile([128, RPC], f16, tag="ql")

        # ---------------- prep ----------------
        with ExitStack() as prep:
            ipool = prep.enter_context(tc.tile_pool(name="ipool", bufs=1))
            pps = prep.enter_context(tc.tile_pool(name="pps", bufs=2, space="PSUM"))

            qh_in = [
                ipool.tile([128, RPC], f16, tag=f"qhin{c}", name=f"qhin{c}")
                for c in range(2)
            ]
            ql_in = [
                ipool.tile([128, RPC], f16, tag=f"qlin{c}", name=f"qlin{c}")
                for c in range(2)
            ]
            ih = [
                ipool.tile([128, N], f16, tag=f"ih{c}", name=f"ih{c}")
                for c in range(2)
            ]
            il = [
                ipool.tile([128, N], f16, tag=f"il{c}", name=f"il{c}")
                for c in range(2)
            ]
            for c in range(2):
                sl = slice(c * 128, (c + 1) * 128)
                nc.sync.dma_start(qh_in[c][:], qih[sl, :])
                nc.sync.dma_start(ql_in[c][:], qil[sl, :])
            # big input DMAs sliced so prep matmuls start early
            DSL = 2048
            for j in range(N // DSL):
                js = slice(j * DSL, (j + 1) * DSL)
                for c in range(2):
                    sl = slice(c * 128, (c + 1) * 128)
                    nc.sync.dma_start(ih[c][:, js], inh[sl, js])
                    nc.sync.dma_start(il[c][:, js], inl[sl, js])

            wslice = lambda c, i: (wp0 if c == 0 else wp1)[:, i * H : (i + 1) * H]

            def hilo3(ps_ap, w_hi_i, w_lo_i, mov_h, mov_l):
                for c in range(2):
                    mm(ps_ap, wslice(c, w_hi_i), mov_h[c], start=(c == 0), stop=False)
                    mm(ps_ap, wslice(c, w_hi_i), mov_l[c], start=False, stop=False)
                    mm(ps_ap, wslice(c, w_lo_i), mov_h[c], start=False,
                       stop=(c == 1))

            # qT -> qh/ql fp16 pair (small; first, while big DMAs stream)
            QB = min(KB, RPC)
            for b in range(RPC // QB):
                rs = slice(b * QB, (b + 1) * QB)
                pq = pps.tile([128, QB], f32, tag="pq")
                hilo3(pq[:], 0, 1, [t[:, rs] for t in qh_in], [t[:, rs] for t in ql_in])
                nc.scalar.copy(qh[:, rs], pq[:])
                nc.vector.tensor_sub(ql[:, rs], pq[:], qh[:, rs])

            # keyT -> kh/kl fp16 pair
            for b in range(NKB):
                ks = slice(b * KB, (b + 1) * KB)
                pk = pps.tile([128, KB], f32, tag="pk")
                hilo3(pk[:], 2, 3, [t[:, ks] for t in ih], [t[:, ks] for t in il])
                nc.scalar.copy(kh[:, ks], pk[:])
                nc.vector.tensor_sub(kl[:, ks], pk[:], kh[:, ks])

            # v fp16 (keys on partitions)
            for b4 in range(NKT // 4):
                pv = pps.tile([128, 512], f32, tag="pv")
                for j in range(4):
                    kt = b4 * 4 + j
                    kslc = slice(kt * 128, (kt + 1) * 128)
                    for c in range(2):
                        mm(
                            pv[:, j * 128 : (j + 1) * 128],
                            ih[c][:, kslc],
                            wslice(c, 4),
                            start=(c == 0),
                            stop=(c == 1),
                            skip_group_check=True,
                        )
                nc.scalar.copy(vsb[:, b4 * 512 : (b4 + 1) * 512], pv[:])

        # ---------------- main ----------------
        # PSUM budget (8 banks): spsum 3x2 + opsum 1 + dpsum 1
        ppool = ctx.enter_context(tc.tile_pool(name="ppool", bufs=2 * GRP))
        smalls = ctx.enter_context(tc.tile_pool(name="smalls", bufs=4))
        ptsb = ctx.enter_context(tc.tile_pool(name="ptsb", bufs=3))
        ostr = ctx.enter_context(tc.tile_pool(name="ostr", bufs=3))
        spsum = ctx.enter_context(tc.tile_pool(name="spsum", bufs=3, space="PSUM"))
        opsum = ctx.enter_context(tc.tile_pool(name="opsum", bufs=1, space="PSUM"))
        dpsum = ctx.enter_context(tc.tile_pool(name="dpsum", bufs=1, space="PSUM"))

        P_tiles = {}
        pts_tiles = {}

        def softmax_tile(rt):
            P = ppool.tile([128, N], f16, tag="P")
            P_tiles[rt] = P
            negB = smalls.tile([128, NCH], f32, tag="negB")
            sums = smalls.tile([128, NCH], f32, tag="sums")
            lh = qh[:, rt * 128 : (rt + 1) * 128]
            ll = ql[:, rt * 128 : (rt + 1) * 128]
            for c in range(NCH):
                ps = spsum.tile([128, CHUNK], f32, tag="ps")
                for hblk in range(CHUNK // KB):
                    o = ps[:, hblk * KB : (hblk + 1) * KB]
                    ks = slice(c * CHUNK + hblk * KB, c * CHUNK + (hblk + 1) * KB)
                    mm(o, lh, kh[:, ks], start=True, stop=False)
                    mm(o, lh, kl[:, ks], start=False, stop=False)
                    mm(o, ll, kh[:, ks], start=False, stop=True)
                nc.vector.tensor_reduce(
                    negB[:, c : c + 1], ps[:], axis=AXX, op=MAX, negate=True
                )
                nc.scalar.activation(
                    P[:, c * CHUNK : (c + 1) * CHUNK],
                    ps[:],
                    EXP,
                    bias=negB[:, c : c + 1],
                    scale=1.0,
                    accum_out=sums[:, c : c + 1],
                )
            negM = smalls.tile([128, 1], f32, tag="negM")
            nc.vector.tensor_reduce(
                negM[:], negB[:], axis=AXX, op=mybir.AluOpType.min
            )
            F = smalls.tile([128, NCH], f32, tag="F")
            nc.scalar.activation(F[:], negB[:], EXP, bias=negM[:], scale=-1.0)
            T = smalls.tile([128, NCH], f32, tag="T")
            nc.vector.tensor_mul(T[:], F[:], sums[:])
            S = smalls.tile([128, 1], f32, tag="S")
            nc.vector.tensor_reduce(S[:], T[:], axis=AXX, op=mybir.AluOpType.add)
            R = smalls.tile([128, 1], f32, tag="R")
            nc.vector.reciprocal(R[:], S[:])
            G = smalls.tile([128, NCH], f32, tag="G")
            nc.vector.tensor_scalar_mul(G[:], F[:], R[:])
            for c in range(NCH):
                sl = slice(c * CHUNK, (c + 1) * CHUNK)
                nc.vector.tensor_scalar_mul(P[:, sl], P[:, sl], G[:, c : c + 1])

        def transpose_group(g):
            """DMA xbar transposes: P^T half-tiles [128, HKT, GRP*128]."""
            for h in range(2):
                pts = ptsb.tile([128, HKT, GRP * 128], f16, tag="pts")
                pts_tiles[(g, h)] = pts
                for j in range(GRP):
                    rt = g * GRP + j
                    nc.sync.dma_start(
                        pts[:][:, :, j * 128 : (j + 1) * 128],
                        P_tiles[rt][:, h * (N // 2) : (h + 1) * (N // 2)],
                        transpose=True,
                    )

        oTs_tiles = {}

        def av_group(g):
            oab = opsum.tile([128, 2 * GRP * 128], f32, tag="oab")
            for h in range(2):
                oacc = oab[:, h * GRP * 128 : (h + 1) * GRP * 128]
                pts = pts_tiles.pop((g, h))
                for i in range(HKT):
                    kt = h * HKT + i
                    mm(
                        oacc,
                        vsb[:, kt * 128 : (kt + 1) * 128],
                        pts[:][:, i, :],
                        start=(i == 0),
                        stop=(i == HKT - 1),
                    )
            oTs = ostr.tile([128, GRP * 128], f32, tag="oTs")
            nc.scalar.copy(oTs[:], oab[:, 0 : GRP * 128])
            nc.vector.tensor_add(
                oTs[:], oTs[:], oab[:, GRP * 128 : 2 * GRP * 128]
            )
            oTs_tiles[g] = oTs

        def de_group(g):
            # emitted late so the pd matmuls never stall the fenced PE stream
            oTs = oTs_tiles.pop(g)
            for j in range(GRP):
                rt = g * GRP + j
                pd = dpsum.tile([128, D], f32, tag="pd")
                mm(pd[:], oTs[:, j * 128 : (j + 1) * 128], vu[:], start=True, stop=True)
                dEs = ostr.tile([128, D], f32, tag="dEs")
                nc.scalar.copy(dEs[:], pd[:])
                nc.sync.dma_start(out[rt * 128 : (rt + 1) * 128, :], dEs[:])

        for rt in range(RT):
            softmax_tile(rt)
            if rt % GRP == GRP - 1:
                g = rt // GRP
                transpose_group(g)
                if g >= 1:
                    av_group(g - 1)
                if g >= 2:
                    de_group(g - 2)
        av_group(NG - 1)
        for g in sorted(oTs_tiles.keys()):
            de_group(g)

    return nc


def _split16(x):
    hi = x.astype(np.float16)
    lo = (x - hi.astype(np.float32)).astype(np.float16)
    return hi, lo


def kernel(input, Q, K, V_down, V_up):
    input = np.asarray(input, np.float32)
    Q = np.asarray(Q, np.float32)
    K = np.asarray(K, np.float32)
    V_down = np.asarray(V_down, np.float32)
    V_up = np.asarray(V_up, np.float32)

    N, D = input.shape
    H = Q.shape[1]
    RPC = N // N_CORES

    inT = np.ascontiguousarray(input.T)  # [D, N]
    inh, inl = _split16(inT)
    Qh, Ql = _split16(Q)
    Kh, Kl = _split16(K)
    Vdh = V_down.astype(np.float16)
    wpk = np.ascontiguousarray(np.concatenate([Qh, Ql, Kh, Kl, Vdh], axis=1))

    nc = build(N=N, D=D, H=H, RPC=RPC)
    nc.finalize()

    in_maps = []
    for c in range(N_CORES):
        sl = slice(c * RPC, (c + 1) * RPC)
        in_maps.append(
            {
                "inh": inh,
                "inl": inl,
                "qih": np.ascontiguousarray(inh[:, sl]),
                "qil": np.ascontiguousarray(inl[:, sl]),
                "wpk": wpk,
                "vup": V_up,
            }
        )

    res = run_bass_kernel_spmd(nc, in_maps, core_ids=list(range(N_CORES)))
    return np.concatenate([res.results[c]["out"] for c in range(N_CORES)], axis=0)
